# revision 49
# baseline (speedup 1.0000x reference)
"""CCPL contrastive-loss kernel for Trainium2 (8 NeuronCores).

Strategy: the loss only touches 256 sampled 3x3 neighborhoods of
feat_q/feat_k (~4.7 MB of each 512 MiB tensor), so the kernel never
streams the full tensors.  Work is data-parallel over the batch dim:
core b receives feat_q[b] / feat_k[b] re-laid-out channel-last in bf16
([H*W, 128] with q on channels 0-63, k on 64-127), so each sampled
pixel's 128 channels are one contiguous 256 B run in HBM and window
rows (3 pixels) are 768 B runs.  The gather runs on the SWDGE indirect
path with one offset per destination partition (the HW ucode's
contract): 6 calls of offsets=[128,1] -> out=[128, 768 B], one per
(sample-slot, window-row).  Samples land on SBUF partitions, channels
on the free axis; per-(sample, position, tensor) L2 norms are free-axis
block reductions on DVE (bf16 in, f32 out), the normalize pipeline is
split across DVE/ACT/Pool, |q_hat - k_hat| is summed by ACT's fused
Abs+accumulate, and the final cross-partition sum is one PE matmul.
The sample indices ship as data (int32 [128, 6]), so the program never
recompiles when sample_ids change.  The host sums the 8 per-core
partials and divides by the element count.
"""

import os
import sys
from contextlib import ExitStack

import numpy as np

sys.path.insert(0, "/opt/trn_rl_repo")

import ml_dtypes

import concourse.bass as bass
import concourse.tile as tile
from concourse import mybir
from concourse.bass_utils import run_bass_kernel_spmd


def _install_ntff_hook():
    """Provide antenv.axon_hooks when the agent image lacks it.

    concourse's axon trace path imports antenv.axon_hooks to fetch the
    NTFF profile hook; this image's antenv has no such submodule.  The
    hook implementation ships in trn_agent_boot.trn_boot, so wire it up
    against the axon PJRT .so directly.
    """
    try:
        from antenv.axon_hooks import get_axon_ntff_profile_hook  # noqa: F401

        return
    except ImportError:
        pass
    import types

    hook = None
    try:
        from trn_agent_boot.trn_boot import _ntff_profile_via_ctypes

        so = "/opt/axon/libaxon_pjrt.so"
        if os.path.exists(so):
            hook = _ntff_profile_via_ctypes(so)
    except Exception:
        hook = None
    mod = types.ModuleType("antenv.axon_hooks")
    _state = {"hook": hook}
    mod.get_axon_ntff_profile_hook = lambda: _state["hook"]
    mod.set_axon_ntff_profile_hook = lambda h: _state.update(hook=h)
    import antenv

    sys.modules["antenv.axon_hooks"] = mod
    antenv.axon_hooks = mod


_install_ntff_hook()

B, C, H, W = 8, 64, 512, 512
NUM_S = 256
P = 2 * C  # q on channels 0-63, k on 64-127 of the channel-last layout
NSLOT = 2  # 256 samples -> 2 slots of 128 (sample s = slot*128 + partition)
N_CORES = 8

_cache: dict = {}
LAST_RESULTS = None  # BassKernelResults of the most recent run (for test.py)


def _split_multi_waits(nc):
    """Walrus build here embeds at most ONE sync wait per instruction.

    Tile emits instructions (notably the kernel-tail Drain) carrying many
    sem waits.  Hoist all but the last wait of any such instruction onto
    single-wait NOPs inserted immediately before it on the same queue —
    the queue stalls on each NOP in turn, preserving semantics.
    """
    from concourse import mybir as _mybir

    for f in nc.m.functions:
        for blk in f.blocks:
            insts = blk.instructions
            i = 0
            while i < len(insts):
                inst = insts[i]
                si = inst.sync_info
                if si is not None and si.on_wait and len(si.on_wait) > 1:
                    waits = list(si.on_wait)
                    si.on_wait = waits[-1:]
                    for j, w in enumerate(waits[:-1]):
                        nop = _mybir.InstNoOp(
                            name=nc.get_next_instruction_name(),
                            ins=[],
                            outs=[],
                            engine=inst.engine,
                            sync_info=_mybir.SyncInfo(on_wait=[w], on_update=[]),
                        )
                        insts.insert(i + j, nop)
                    i += len(waits) - 1
                i += 1


def _build(split_waits=True):
    f32 = mybir.dt.float32
    bf16 = mybir.dt.bfloat16
    i32 = mybir.dt.int32
    sub = mybir.AluOpType.subtract
    mul = mybir.AluOpType.mult
    add = mybir.AluOpType.add
    nc = bass.Bass()

    # channel-last stacked features in bf16: row (h*512 + w) holds the
    # 128 q|k channels of pixel (h, w) as one contiguous 256 B run.
    fqkT = nc.dram_tensor("fqkT", [H * W, P], bf16, kind="ExternalInput")
    # idx[p, t*3 + r] = (h+r)*512 + w for sample s = t*128 + p.  The HW
    # indirect-DMA ucode consumes exactly ONE offset per destination
    # partition (multi-offset APs silently misgather), so the window
    # gather is 6 calls of shape offsets=[128,1] -> out=[128, 3*128]
    # (rows w..w+2 are contiguous channel-last, 768 B per partition).
    idxT = nc.dram_tensor("idx", [128, NSLOT * 3], i32, kind="ExternalInput")
    out = nc.dram_tensor("out", [4, 1], f32, kind="ExternalOutput")

    with tile.TileContext(nc) as tc, ExitStack() as ctx:
        sb = ctx.enter_context(tc.tile_pool(name="sb", bufs=1))
        pf = ctx.enter_context(tc.tile_pool(name="pf", bufs=1, space="PSUM"))

        idx = sb.tile([128, NSLOT * 3], i32)
        nc.sync.dma_start(out=idx[:], in_=idxT[:])

        ones = sb.tile([128, 1], f32)
        nc.vector.memset(ones[:], 1.0)
        # engine warmups: PE clock + ACT sqrt-table load happen off the
        # critical path while the index table streams in.
        warm = pf.tile([1, 1], f32, tag="warm")
        nc.tensor.matmul(out=warm[:], lhsT=ones[:], rhs=ones[:], start=True, stop=True)
        actw = sb.tile([128, 1], f32)
        nc.scalar.sqrt(out=actw[:], in_=ones[:])
        tiny = sb.tile([128, 1], f32)
        nc.vector.memset(tiny[:], 1e-14)

        qk = sb.tile([128, NSLOT, 9, P], bf16)  # gathered windows
        # slot 1's scratch aliases slot 0's (d <-> d2 swapped): the WAR
        # dependencies force the list scheduler to emit slot 0's reduce
        # and normalize before slot 1's subtract/square on DVE — without
        # them it reorders on its (optimistic) DMA timing model and
        # strands DVE idle behind slot 1's gather.
        dA = sb.tile([128, 9, P], bf16)  # slot0: window-center
        dB = sb.tile([128, 9, P], bf16)  # slot0: d^2 | slot1: window-center
        dC = sb.tile([128, 9, P], bf16)  # slot1: d^2
        xh = sb.tile([128, NSLOT, 9, P], bf16)  # normalized (q_hat | k_hat)
        n2 = sb.tile([128, NSLOT, 18], bf16)  # block B = j*2 + (0:q, 1:k)
        nrm = sb.tile([128, NSLOT, 18], bf16)
        rinv = sb.tile([128, NSLOT, 18], bf16)
        dif = sb.tile([128, NSLOT, 9, C], bf16)
        difa = sb.tile([128, 9 * C], bf16)
        acc = sb.tile([128, 4], f32)  # (slot, rows01|row2) partials

        # 6 SWDGE gathers (slot-major so slot 0 lands first): each brings
        # one window row (3 positions x 128 ch, 768 B) for 128 samples.
        qkr = qk[:].rearrange("p t (r dw) c -> p t r (dw c)", r=3)
        for t in range(NSLOT):
            for r in range(3):
                nc.gpsimd.indirect_dma_start(
                    out=qkr[:, t, r],
                    out_offset=None,
                    in_=fqkT[:],
                    in_offset=bass.IndirectOffsetOnAxis(
                        ap=idx[:, t * 3 + r : t * 3 + r + 1], axis=0
                    ),
                )

        def slot_compute(t, _lp=nc.allow_low_precision):
            lp = ctx.enter_context(
                _lp("bf16 norm pipeline: ~2e-3 rel on per-column norms is "
                    "far inside the 2e-2 loss gate (measured 4e-4 overall)")
            )
            d = dA if t == 0 else dB
            d2 = dB if t == 0 else dC
            # center-subtract and square start as soon as window rows 0-1
            # land (position blocks j = r*3+dw, center at j=4 is in row 1);
            # the row-2 parts follow when the third gather completes.
            ctr = qk[:, t, 4:5, :]
            nc.vector.tensor_tensor(
                out=d[:, 0:6], in0=qk[:, t, 0:6],
                in1=ctr.to_broadcast([128, 6, P]), op=sub,
            )
            nc.vector.tensor_tensor(
                out=d2[:, 0:6], in0=d[:, 0:6], in1=d[:, 0:6], op=mul
            )
            db = d[:].rearrange("p j (b c) -> p (j b) c", b=2)
            d2b = d2[:].rearrange("p j (b c) -> p (j b) c", b=2)
            xb = xh[:, t].rearrange("p j (b c) -> p (j b) c", b=2)
            xq = xh[:, t].rearrange("p j (b c) -> p j b c", b=2)

            def norm_tail(bs, be, acc_col, dif_eng, red_eng):
                # blocks [bs, be) -> norms, normalize, q_hat-k_hat, abs-sum
                nB = be - bs
                nc.vector.tensor_reduce(
                    out=n2[:, t, bs:be], in_=d2b[:, bs:be],
                    axis=mybir.AxisListType.X, op=add,
                )
                # rinv = 1/sqrt(n2 + tiny); center block n2=0 -> d=0 -> 0
                nc.scalar.activation(
                    out=nrm[:, t, bs:be], in_=n2[:, t, bs:be],
                    func=mybir.ActivationFunctionType.Sqrt, bias=tiny[:],
                )
                nc.vector.reciprocal(out=rinv[:, t, bs:be], in_=nrm[:, t, bs:be])
                rb = rinv[:, t, bs:be].unsqueeze(2).to_broadcast([128, nB, C])
                nc.vector.tensor_tensor(
                    out=xb[:, bs:be], in0=db[:, bs:be], in1=rb, op=mul
                )
                js, je = bs // 2, be // 2
                dif_eng.tensor_tensor(
                    out=dif[:, t, js:je], in0=xq[:, js:je, 0],
                    in1=xq[:, js:je, 1], op=sub,
                )
                if red_eng is nc.vector:
                    nc.vector.tensor_reduce(
                        out=acc[:, acc_col : acc_col + 1],
                        in_=dif[:, t, js:je].rearrange("p j c -> p (j c)"),
                        axis=mybir.AxisListType.X,
                        op=add,
                        apply_absolute_value=True,
                    )
                else:
                    nc.scalar.activation(
                        out=difa[:, js * C : je * C],
                        in_=dif[:, t, js:je].rearrange("p j c -> p (j c)"),
                        func=mybir.ActivationFunctionType.Abs,
                        accum_out=acc[:, acc_col : acc_col + 1],
                    )

            # Both slots split at the row-2 boundary: blocks 0-11 depend
            # only on window rows 0-1 and compute while later gathers are
            # in flight; only the block 12-17 partials chain behind the
            # slot's last gather.  Only slot 1's row-2 part (the very end
            # of the critical path) keeps its q_hat-k_hat / abs-sum on
            # DVE; everything else goes to Pool + ACT.
            norm_tail(0, 12, 2 * t, nc.gpsimd, nc.scalar)
            nc.vector.tensor_tensor(
                out=d[:, 6:9], in0=qk[:, t, 6:9],
                in1=ctr.to_broadcast([128, 3, P]), op=sub,
            )
            # ACT takes the small row-2 square so DVE can run straight
            # into the reduce
            nc.scalar.square(out=d2[:, 6:9], in_=d[:, 6:9])
            if t == 0:
                norm_tail(12, 18, 2 * t + 1, nc.gpsimd, nc.scalar)
            else:
                norm_tail(12, 18, 2 * t + 1, nc.vector, nc.vector)

        # slot 0's chain is scheduled at elevated priority so the list
        # scheduler never hoists slot 1's bulk ops ahead of slot 0's tiny
        # norm ops on ACT (observed +5us critical-path cost).
        with tc.high_priority(offset=64):
            slot_compute(0)
        slot_compute(1)

        # cross-partition sum: out[t] = sum_p acc[p, t].  One PE matmul +
        # a single 8 B DRAM write beats DMAing acc[128, 2] (128 scattered
        # 8 B HBM writes each pay a read-modify-write round trip).
        pfin = pf.tile([4, 1], f32, tag="fin")
        nc.tensor.matmul(out=pfin[:], lhsT=acc[:], rhs=ones[:], start=True, stop=True)
        res = sb.tile([4, 1], f32)
        nc.scalar.copy(out=res[:], in_=pfin[:])
        nc.sync.dma_start(out=out[:], in_=res[:])

    if split_waits:
        _split_multi_waits(nc)
    return nc


def kernel(feat_q, feat_k, sample_ids, *, trace=False, trace_cores=None):
    global LAST_RESULTS
    feat_q = np.asarray(feat_q, dtype=np.float32)
    feat_k = np.asarray(feat_k, dtype=np.float32)
    ids = np.asarray(sample_ids).astype(np.int64)

    if "prog" not in _cache:
        _cache["prog"] = _build()
    nc = _cache["prog"]

    # idx[p, t*3 + r] = flat position of window row r for sample t*128 + p
    hs, ws = ids[:, 0], ids[:, 1]
    r = np.arange(3)
    rowpos = (hs[:, None] + r[None, :]) * W + ws[:, None]  # [256, 3]
    idx = np.ascontiguousarray(
        rowpos.reshape(NSLOT, 128, 3).transpose(1, 0, 2).reshape(128, NSLOT * 3)
    ).astype(np.int32)

    in_maps = []
    for b in range(N_CORES):
        fqk = np.concatenate([feat_q[b], feat_k[b]], axis=0)  # [128, H, W]
        fqkT = np.ascontiguousarray(fqk.transpose(1, 2, 0)).reshape(H * W, P)
        in_maps.append({"fqkT": fqkT.astype(ml_dtypes.bfloat16), "idx": idx})

    results = run_bass_kernel_spmd(
        nc,
        in_maps,
        core_ids=list(range(N_CORES)),
        trace=trace,
        trace_cores=trace_cores,
    )
    LAST_RESULTS = results
    total = np.float64(0.0)
    for r_ in results.results:
        total += np.float64(r_["out"].astype(np.float64).sum())
    loss = total / (B * C * 8 * NUM_S)
    return np.asarray(loss, dtype=np.float32)


# revision 51
# speedup vs baseline: 1.2011x; 1.2011x over previous
"""CCPL contrastive-loss kernel for Trainium2 (8 NeuronCores).

Strategy: the loss only touches 256 sampled 3x3 neighborhoods of
feat_q/feat_k (~4.7 MB of each 512 MiB tensor), so the kernel never
streams the full tensors.  Work is data-parallel over the batch dim:
core b receives feat_q[b] / feat_k[b] re-laid-out channel-last in bf16
([H*W, 128] with q on channels 0-63, k on 64-127), so each sampled
pixel's 128 channels are one contiguous 256 B run in HBM and window
rows (3 pixels) are 768 B runs.  The gather runs on the SWDGE indirect
path with one offset per destination partition (the HW ucode's
contract): 6 calls of offsets=[128,1] -> out=[128, 768 B], one per
(sample-slot, window-row).  Samples land on SBUF partitions, channels
on the free axis; per-(sample, position, tensor) L2 norms are free-axis
block reductions on DVE (bf16 in, f32 out), the normalize pipeline is
split across DVE/ACT/Pool, |q_hat - k_hat| is summed by ACT's fused
Abs+accumulate, and the final cross-partition sum is one PE matmul.
The sample indices ship as data (int32 [128, 6]), so the program never
recompiles when sample_ids change.  The host sums the 8 per-core
partials and divides by the element count.
"""

import os
import sys
from contextlib import ExitStack

import numpy as np

sys.path.insert(0, "/opt/trn_rl_repo")

import ml_dtypes

import concourse.bass as bass
import concourse.tile as tile
from concourse import mybir
from concourse.bass_utils import run_bass_kernel_spmd


def _install_ntff_hook():
    """Provide antenv.axon_hooks when the agent image lacks it.

    concourse's axon trace path imports antenv.axon_hooks to fetch the
    NTFF profile hook; this image's antenv has no such submodule.  The
    hook implementation ships in trn_agent_boot.trn_boot, so wire it up
    against the axon PJRT .so directly.
    """
    try:
        from antenv.axon_hooks import get_axon_ntff_profile_hook  # noqa: F401

        return
    except ImportError:
        pass
    import types

    hook = None
    try:
        from trn_agent_boot.trn_boot import _ntff_profile_via_ctypes

        so = "/opt/axon/libaxon_pjrt.so"
        if os.path.exists(so):
            hook = _ntff_profile_via_ctypes(so)
    except Exception:
        hook = None
    mod = types.ModuleType("antenv.axon_hooks")
    _state = {"hook": hook}
    mod.get_axon_ntff_profile_hook = lambda: _state["hook"]
    mod.set_axon_ntff_profile_hook = lambda h: _state.update(hook=h)
    import antenv

    sys.modules["antenv.axon_hooks"] = mod
    antenv.axon_hooks = mod


_install_ntff_hook()

B, C, H, W = 8, 64, 512, 512
NUM_S = 256
P = 2 * C  # q on channels 0-63, k on 64-127 of the channel-last layout
NSLOT = 2  # 256 samples -> 2 slots of 128 (sample s = slot*128 + partition)
N_CORES = 8

_cache: dict = {}
LAST_RESULTS = None  # BassKernelResults of the most recent run (for test.py)


def _split_multi_waits(nc):
    """Walrus build here embeds at most ONE sync wait per instruction.

    Tile emits instructions (notably the kernel-tail Drain) carrying many
    sem waits.  Hoist all but the last wait of any such instruction onto
    single-wait NOPs inserted immediately before it on the same queue —
    the queue stalls on each NOP in turn, preserving semantics.
    """
    from concourse import mybir as _mybir

    for f in nc.m.functions:
        for blk in f.blocks:
            insts = blk.instructions
            i = 0
            while i < len(insts):
                inst = insts[i]
                si = inst.sync_info
                if si is not None and si.on_wait and len(si.on_wait) > 1:
                    waits = list(si.on_wait)
                    si.on_wait = waits[-1:]
                    for j, w in enumerate(waits[:-1]):
                        nop = _mybir.InstNoOp(
                            name=nc.get_next_instruction_name(),
                            ins=[],
                            outs=[],
                            engine=inst.engine,
                            sync_info=_mybir.SyncInfo(on_wait=[w], on_update=[]),
                        )
                        insts.insert(i + j, nop)
                    i += len(waits) - 1
                i += 1


def _build(split_waits=True):
    f32 = mybir.dt.float32
    bf16 = mybir.dt.bfloat16
    i32 = mybir.dt.int32
    sub = mybir.AluOpType.subtract
    mul = mybir.AluOpType.mult
    add = mybir.AluOpType.add
    nc = bass.Bass()

    # channel-last stacked features in bf16: row (h*512 + w) holds the
    # 128 q|k channels of pixel (h, w) as one contiguous 256 B run.
    fqkT = nc.dram_tensor("fqkT", [H * W, P], bf16, kind="ExternalInput")
    # idx[p, t*3 + r] = (h+r)*512 + w for sample s = t*128 + p.  The HW
    # indirect-DMA ucode consumes exactly ONE offset per destination
    # partition (multi-offset APs silently misgather), so the window
    # gather is 6 calls of shape offsets=[128,1] -> out=[128, 3*128]
    # (rows w..w+2 are contiguous channel-last, 768 B per partition).
    idxT = nc.dram_tensor("idx", [128, NSLOT * 3], i32, kind="ExternalInput")
    out = nc.dram_tensor("out", [4, 1], f32, kind="ExternalOutput")

    with tile.TileContext(nc) as tc, ExitStack() as ctx:
        sb = ctx.enter_context(tc.tile_pool(name="sb", bufs=1))
        pf = ctx.enter_context(tc.tile_pool(name="pf", bufs=1, space="PSUM"))

        idx = sb.tile([128, NSLOT * 3], i32)
        nc.sync.dma_start(out=idx[:], in_=idxT[:])

        ones = sb.tile([128, 1], f32)
        nc.vector.memset(ones[:], 1.0)
        # engine warmups: PE clock + ACT sqrt-table load happen off the
        # critical path while the index table streams in.
        warm = pf.tile([1, 1], f32, tag="warm")
        nc.tensor.matmul(out=warm[:], lhsT=ones[:], rhs=ones[:], start=True, stop=True)
        actw = sb.tile([128, 1], f32)
        nc.scalar.sqrt(out=actw[:], in_=ones[:])
        tiny = sb.tile([128, 1], f32)
        nc.vector.memset(tiny[:], 1e-14)

        qk = sb.tile([128, NSLOT, 9, P], bf16)  # gathered windows
        # slot 1's scratch aliases slot 0's (d <-> d2 swapped): the WAR
        # dependencies force the list scheduler to emit slot 0's reduce
        # and normalize before slot 1's subtract/square on DVE — without
        # them it reorders on its (optimistic) DMA timing model and
        # strands DVE idle behind slot 1's gather.
        dA = sb.tile([128, 9, P], bf16)  # slot0: window-center
        dB = sb.tile([128, 9, P], bf16)  # slot0: d^2 | slot1: window-center
        dC = sb.tile([128, 9, P], bf16)  # slot1: d^2
        xh = sb.tile([128, NSLOT, 9, P], bf16)  # normalized (q_hat | k_hat)
        n2 = sb.tile([128, NSLOT, 18], bf16)  # block B = j*2 + (0:q, 1:k)
        nrm = sb.tile([128, NSLOT, 18], bf16)
        rinv = sb.tile([128, NSLOT, 18], bf16)
        dif = sb.tile([128, NSLOT, 9, C], bf16)
        difa = sb.tile([128, 9 * C], bf16)
        acc = sb.tile([128, 4], f32)  # (slot, rows01|row2) partials

        # 6 SWDGE gathers (slot-major so slot 0 lands first): each brings
        # one window row (3 positions x 128 ch, 768 B) for 128 samples.
        qkr = qk[:].rearrange("p t (r dw) c -> p t r (dw c)", r=3)
        for t in range(NSLOT):
            for r in range(3):
                nc.gpsimd.indirect_dma_start(
                    out=qkr[:, t, r],
                    out_offset=None,
                    in_=fqkT[:],
                    in_offset=bass.IndirectOffsetOnAxis(
                        ap=idx[:, t * 3 + r : t * 3 + r + 1], axis=0
                    ),
                )

        def slot_compute(t, _lp=nc.allow_low_precision):
            lp = ctx.enter_context(
                _lp("bf16 norm pipeline: ~2e-3 rel on per-column norms is "
                    "far inside the 2e-2 loss gate (measured 4e-4 overall)")
            )
            d = dA if t == 0 else dB
            d2 = dB if t == 0 else dC
            # center-subtract and square start as soon as window rows 0-1
            # land (position blocks j = r*3+dw, center at j=4 is in row 1);
            # the row-2 parts follow when the third gather completes.
            ctr = qk[:, t, 4:5, :]
            nc.vector.tensor_tensor(
                out=d[:, 0:6], in0=qk[:, t, 0:6],
                in1=ctr.to_broadcast([128, 6, P]), op=sub,
            )
            nc.vector.tensor_tensor(
                out=d2[:, 0:6], in0=d[:, 0:6], in1=d[:, 0:6], op=mul
            )
            db = d[:].rearrange("p j (b c) -> p (j b) c", b=2)
            d2b = d2[:].rearrange("p j (b c) -> p (j b) c", b=2)
            xb = xh[:, t].rearrange("p j (b c) -> p (j b) c", b=2)
            xq = xh[:, t].rearrange("p j (b c) -> p j b c", b=2)

            def norm_tail(bs, be, acc_col, dif_eng, red_eng):
                # blocks [bs, be) -> norms, normalize, q_hat-k_hat, abs-sum
                nB = be - bs
                nc.vector.tensor_reduce(
                    out=n2[:, t, bs:be], in_=d2b[:, bs:be],
                    axis=mybir.AxisListType.X, op=add,
                )
                # rinv = 1/sqrt(n2 + tiny); center block n2=0 -> d=0 -> 0
                nc.scalar.activation(
                    out=nrm[:, t, bs:be], in_=n2[:, t, bs:be],
                    func=mybir.ActivationFunctionType.Sqrt, bias=tiny[:],
                )
                nc.vector.reciprocal(out=rinv[:, t, bs:be], in_=nrm[:, t, bs:be])
                rb = rinv[:, t, bs:be].unsqueeze(2).to_broadcast([128, nB, C])
                nc.vector.tensor_tensor(
                    out=xb[:, bs:be], in0=db[:, bs:be], in1=rb, op=mul
                )
                js, je = bs // 2, be // 2
                dif_eng.tensor_tensor(
                    out=dif[:, t, js:je], in0=xq[:, js:je, 0],
                    in1=xq[:, js:je, 1], op=sub,
                )
                if red_eng is nc.vector:
                    nc.vector.tensor_reduce(
                        out=acc[:, acc_col : acc_col + 1],
                        in_=dif[:, t, js:je].rearrange("p j c -> p (j c)"),
                        axis=mybir.AxisListType.X,
                        op=add,
                        apply_absolute_value=True,
                    )
                else:
                    nc.scalar.activation(
                        out=difa[:, js * C : je * C],
                        in_=dif[:, t, js:je].rearrange("p j c -> p (j c)"),
                        func=mybir.ActivationFunctionType.Abs,
                        accum_out=acc[:, acc_col : acc_col + 1],
                    )

            # Both slots split at the row-2 boundary: blocks 0-11 depend
            # only on window rows 0-1 and compute while later gathers are
            # in flight; only the block 12-17 partials chain behind the
            # slot's last gather.  Slot 0's q_hat-k_hat / abs-sum go to
            # Pool + ACT (off the critical path); slot 1's stay on DVE.
            dif_eng = nc.gpsimd if t == 0 else nc.vector
            red_eng = nc.scalar if t == 0 else nc.vector
            norm_tail(0, 12, 2 * t, dif_eng, red_eng)
            nc.vector.tensor_tensor(
                out=d[:, 6:9], in0=qk[:, t, 6:9],
                in1=ctr.to_broadcast([128, 3, P]), op=sub,
            )
            # ACT takes the small row-2 square so DVE can run straight
            # into the reduce
            nc.scalar.square(out=d2[:, 6:9], in_=d[:, 6:9])
            norm_tail(12, 18, 2 * t + 1, dif_eng, red_eng)

        # slot 0's chain is scheduled at elevated priority so the list
        # scheduler never hoists slot 1's bulk ops ahead of slot 0's tiny
        # norm ops on ACT (observed +5us critical-path cost).
        with tc.high_priority(offset=64):
            slot_compute(0)
        slot_compute(1)

        # cross-partition sum: out[t] = sum_p acc[p, t].  One PE matmul +
        # a single 8 B DRAM write beats DMAing acc[128, 2] (128 scattered
        # 8 B HBM writes each pay a read-modify-write round trip).
        pfin = pf.tile([4, 1], f32, tag="fin")
        nc.tensor.matmul(out=pfin[:], lhsT=acc[:], rhs=ones[:], start=True, stop=True)
        res = sb.tile([4, 1], f32)
        nc.scalar.copy(out=res[:], in_=pfin[:])
        nc.sync.dma_start(out=out[:], in_=res[:])

    if split_waits:
        _split_multi_waits(nc)
    return nc


def kernel(feat_q, feat_k, sample_ids, *, trace=False, trace_cores=None):
    global LAST_RESULTS
    feat_q = np.asarray(feat_q, dtype=np.float32)
    feat_k = np.asarray(feat_k, dtype=np.float32)
    ids = np.asarray(sample_ids).astype(np.int64)

    if "prog" not in _cache:
        _cache["prog"] = _build()
    nc = _cache["prog"]

    # idx[p, t*3 + r] = flat position of window row r for sample t*128 + p
    hs, ws = ids[:, 0], ids[:, 1]
    r = np.arange(3)
    rowpos = (hs[:, None] + r[None, :]) * W + ws[:, None]  # [256, 3]
    idx = np.ascontiguousarray(
        rowpos.reshape(NSLOT, 128, 3).transpose(1, 0, 2).reshape(128, NSLOT * 3)
    ).astype(np.int32)

    in_maps = []
    for b in range(N_CORES):
        fqk = np.concatenate([feat_q[b], feat_k[b]], axis=0)  # [128, H, W]
        fqkT = np.ascontiguousarray(fqk.transpose(1, 2, 0)).reshape(H * W, P)
        in_maps.append({"fqkT": fqkT.astype(ml_dtypes.bfloat16), "idx": idx})

    results = run_bass_kernel_spmd(
        nc,
        in_maps,
        core_ids=list(range(N_CORES)),
        trace=trace,
        trace_cores=trace_cores,
    )
    LAST_RESULTS = results
    total = np.float64(0.0)
    for r_ in results.results:
        total += np.float64(r_["out"].astype(np.float64).sum())
    loss = total / (B * C * 8 * NUM_S)
    return np.asarray(loss, dtype=np.float32)
